# revision 4
# baseline (speedup 1.0000x reference)
"""CrossCorrelationFFT kernel for 8 Trainium2 NeuronCores.

Math: for x[B=4, H=256, W=256, C=32]:
  - per-(b,c) spatial standardization (mean 0, pop-std 1, scaled 1/sqrt(N))
  - circular cross-correlation of all 528 channel pairs (i<=j) via FFT,
    evaluated only at the 21x21 shift window, out [B, 21, 21, 528] f32.

Device mapping (SPMD, one program, per-core data):
  core k -> b = k//2, ky-half = k%2. Each core:
    - row rDFT along W as matmuls (Y = X @ [Cw | -Sw])
    - column DFT restricted to its 128-ky half via per-core DFT constants
      (F = CrT_h^T @ Ya + SrT_h^T @ Yb), so the pair contraction over ky
      splits across core pairs and the host just adds the two partials.
    - per pair: cross-spectrum (DVE) + partial inverse-DFT contractions (PE)
      against E (ky->sy) and Wk (kx->sx) matrices.
Host: standardize, build DFT constants, sum ky-half partials, transpose.
"""

import sys

import numpy as np

B, H, W, C = 4, 256, 256, 32
MS = 10
NS = 2 * MS + 1  # 21
KX = W // 2 + 1  # 129
NPIX = H * W
P = C * (C + 1) // 2  # 528
STD_EPS = 1e-9


def _standardize(x):
    xc = x - x.mean(axis=(1, 2), keepdims=True)
    stds = xc.std(axis=(1, 2), keepdims=True)
    stds = np.where(stds < STD_EPS, np.inf, stds)
    return (xc / (stds * np.sqrt(np.float32(NPIX)))).astype(np.float32)


def _host_consts():
    w_ = np.arange(W)
    kx = np.arange(KX)
    ang = 2 * np.pi * np.outer(w_, kx) / W  # [W, KX]
    cwt = np.concatenate([np.cos(ang), -np.sin(ang)], axis=1).astype(np.float32)

    y = np.arange(H)
    consts = []
    for half in range(2):
        ky = half * 128 + np.arange(128)
        a2 = 2 * np.pi * np.outer(y, ky) / H  # [H, 128]
        crt = np.cos(a2).astype(np.float32)
        srt = np.sin(a2).astype(np.float32)
        sy = np.arange(NS) - MS
        a3 = 2 * np.pi * np.outer(ky, sy) / H  # [128, NS]
        ert = np.cos(a3).astype(np.float32)
        eit = np.sin(a3).astype(np.float32)
        consts.append((crt, srt, ert, eit, -eit))

    sx = np.arange(NS) - MS
    wgt = np.full(KX, 2.0)
    wgt[0] = 1.0
    wgt[KX - 1] = 1.0
    a4 = 2 * np.pi * np.outer(kx, sx) / W  # [KX, NS]
    wkr = (wgt[:, None] * np.cos(a4) / NPIX).astype(np.float32)
    wkin = (-wgt[:, None] * np.sin(a4) / NPIX).astype(np.float32)
    return cwt, consts, wkr, wkin


def _build_program():
    import concourse.bass as bass
    import concourse.mybir as mybir
    import concourse.tile as tile
    from concourse.vector_clock import ScopedClock, VectorClock

    # walrus rejects sem waits on the SP Drain (TPB_CTRL "Too many sync wait
    # commands"): emit the final waits on NOPs (one proc each), bare drain.
    def _patched_drain_and_barrier(self, tick_clock, wait_clock):
        gc = tick_clock.global_clock
        nprocs = len(list(gc))
        for p in range(nprocs):
            t = gc[p]
            if t > 0:
                nop_inst = self.nc.sync.nop(nofuse=True)
                masked = VectorClock([t if q == p else 0 for q in range(nprocs)])
                wait_clock.add_sem_waits(nop_inst.ins, ScopedClock({None: masked}))
        self.nc.sync.drain()
        self.nc.all_engine_barrier()
        assert self.sems is not None
        popped = self.nc._tile_sem_poison_stack.pop()
        assert popped is self._sem_poison
        self.nc.clear_and_free_semaphores(list(self.sems.allocated().values()))
        self.nc.all_engine_barrier()

    tile.TileContext._drain_and_barrier = _patched_drain_and_barrier

    f32 = mybir.dt.float32
    nc = bass.Bass("TRN2")
    d_xt = nc.dram_tensor("xt", [C, W, H], f32, kind="ExternalInput").ap()
    d_cwt = nc.dram_tensor("cwt", [W, 2 * KX], f32, kind="ExternalInput").ap()
    d_crt = nc.dram_tensor("crt", [H, 128], f32, kind="ExternalInput").ap()
    d_srt = nc.dram_tensor("srt", [H, 128], f32, kind="ExternalInput").ap()
    d_ert = nc.dram_tensor("ert", [128, NS], f32, kind="ExternalInput").ap()
    d_eit = nc.dram_tensor("eit", [128, NS], f32, kind="ExternalInput").ap()
    d_eitn = nc.dram_tensor("eitn", [128, NS], f32, kind="ExternalInput").ap()
    d_wkr = nc.dram_tensor("wkr", [KX, NS], f32, kind="ExternalInput").ap()
    d_wkin = nc.dram_tensor("wkin", [KX, NS], f32, kind="ExternalInput").ap()
    d_out = nc.dram_tensor("yout", [NS, P * NS], f32, kind="ExternalOutput").ap()

    W2 = 2 * KX  # 258
    ii, jj = np.triu_indices(C)

    with tile.TileContext(nc) as tc:
        with (
            tc.tile_pool(name="const", bufs=1) as cpool,
            tc.tile_pool(name="fstore", bufs=1) as fpool,
            tc.tile_pool(name="stage", bufs=1) as spool,
        ):
            t_cwt = [cpool.tile([128, W2], f32, tag=f"cwt{k}", name=f"cwt{k}") for k in range(2)]
            t_crt = [cpool.tile([128, 128], f32, tag=f"crt{k}", name=f"crt{k}") for k in range(2)]
            t_srt = [cpool.tile([128, 128], f32, tag=f"srt{k}", name=f"srt{k}") for k in range(2)]
            t_ert = cpool.tile([128, NS], f32, tag="ert")
            t_eit = cpool.tile([128, NS], f32, tag="eit")
            t_eitn = cpool.tile([128, NS], f32, tag="eitn")
            t_wkr0 = cpool.tile([128, NS], f32, tag="wkr0")
            t_wkr1 = cpool.tile([1, NS], f32, tag="wkr1")
            t_wkin0 = cpool.tile([128, NS], f32, tag="wkin0")
            t_wkin1 = cpool.tile([1, NS], f32, tag="wkin1")
            for k in range(2):
                nc.sync.dma_start(t_cwt[k][:], d_cwt[k * 128 : (k + 1) * 128, :])
                nc.sync.dma_start(t_crt[k][:], d_crt[k * 128 : (k + 1) * 128, :])
                nc.sync.dma_start(t_srt[k][:], d_srt[k * 128 : (k + 1) * 128, :])
            nc.sync.dma_start(t_ert[:], d_ert[:])
            nc.sync.dma_start(t_eit[:], d_eit[:])
            nc.sync.dma_start(t_eitn[:], d_eitn[:])
            nc.sync.dma_start(t_wkr0[:], d_wkr[0:128, :])
            nc.sync.dma_start(t_wkr1[:], d_wkr[128:129, :])
            nc.sync.dma_start(t_wkin0[:], d_wkin[0:128, :])
            nc.sync.dma_start(t_wkin1[:], d_wkin[128:129, :])

            # F storage: per channel c cols [c*W2 : c*W2+129]=Fr, +129..=Fi
            t_F = fpool.tile([128, C * W2], f32, tag="F")
            t_stage = spool.tile([NS, P * NS], f32, tag="st")

            # ---------- FFT stage ----------
            with (
                tc.tile_pool(name="xin", bufs=3) as xpool,
                tc.tile_pool(name="ysb", bufs=3) as ypool,
                tc.tile_pool(name="psA", bufs=2, space="PSUM") as psA,
                tc.tile_pool(name="psB", bufs=2, space="PSUM") as psB,
            ):
                for c in range(C):
                    xts = [xpool.tile([128, H], f32, tag=f"x{k}", name=f"x{k}") for k in range(2)]
                    for k in range(2):
                        nc.sync.dma_start(
                            xts[k][:], d_xt[c, k * 128 : (k + 1) * 128, :]
                        )
                    pf = psB.tile([128, W2], f32, tag="pf")
                    for hb in range(2):  # output row block of Y (H dim)
                        py = psA.tile([128, W2], f32, tag="py")
                        for k in range(2):  # contraction over W
                            nc.tensor.matmul(
                                py[:],
                                xts[k][:, hb * 128 : (hb + 1) * 128],
                                t_cwt[k][:],
                                start=(k == 0),
                                stop=(k == 1),
                            )
                        ya = ypool.tile([128, W2], f32, tag=f"ya{hb}")
                        yb = ypool.tile([128, W2], f32, tag=f"yb{hb}")
                        nc.vector.tensor_copy(ya[:], py[:])
                        nc.vector.tensor_copy(yb[:, 0:KX], py[:, KX:W2])
                        nc.scalar.mul(yb[:, KX:W2], py[:, 0:KX], -1.0)
                        # accumulate F over both H blocks
                        nc.tensor.matmul(
                            pf[:], t_crt[hb][:], ya[:],
                            start=(hb == 0), stop=False,
                        )
                        nc.tensor.matmul(
                            pf[:], t_srt[hb][:], yb[:],
                            start=False, stop=(hb == 1),
                        )
                    nc.vector.tensor_copy(t_F[:, c * W2 : (c + 1) * W2], pf[:])

            # ---------- pair stage ----------
            with (
                tc.tile_pool(name="qt", bufs=3) as qpool,
                tc.tile_pool(name="tt", bufs=3) as tpool,
                tc.tile_pool(name="psT", bufs=2, space="PSUM") as psT,
                tc.tile_pool(name="psT1", bufs=2, space="PSUM") as psT1,
                tc.tile_pool(name="psO", bufs=2, space="PSUM") as psO,
            ):
                for p in range(P):
                    i, j = int(ii[p]), int(jj[p])
                    fri = t_F[:, i * W2 : i * W2 + KX]
                    fii = t_F[:, i * W2 + KX : (i + 1) * W2]
                    frj = t_F[:, j * W2 : j * W2 + KX]
                    fij = t_F[:, j * W2 + KX : (j + 1) * W2]
                    t1 = qpool.tile([128, KX], f32, tag="t1")
                    t2 = qpool.tile([128, KX], f32, tag="t2")
                    q = qpool.tile([128, W2], f32, tag="q")
                    nc.vector.tensor_mul(t1[:], fri, frj)
                    nc.vector.tensor_mul(t2[:], fii, fij)
                    nc.vector.tensor_add(q[:, 0:KX], t1[:], t2[:])
                    nc.vector.tensor_mul(t1[:], fii, frj)
                    nc.vector.tensor_mul(t2[:], fri, fij)
                    nc.vector.tensor_sub(q[:, KX:W2], t1[:], t2[:])

                    q1m = q[:, 0:128]
                    q1n = q[:, 128:129]
                    q2m = q[:, KX : KX + 128]
                    q2n = q[:, KX + 128 : W2]
                    ptt = psT.tile([128, 2 * NS], f32, tag="ptt")
                    pt1 = psT1.tile([1, 2 * NS], f32, tag="pt1")
                    # TrT = Q1^T@ert + Q2^T@eitn ; TiT = Q2^T@ert + Q1^T@eit
                    nc.tensor.matmul(ptt[:, 0:NS], q1m, t_ert[:], start=True, stop=False)
                    nc.tensor.matmul(ptt[:, 0:NS], q2m, t_eitn[:], start=False, stop=True)
                    nc.tensor.matmul(ptt[:, NS:], q2m, t_ert[:], start=True, stop=False)
                    nc.tensor.matmul(ptt[:, NS:], q1m, t_eit[:], start=False, stop=True)
                    nc.tensor.matmul(pt1[:, 0:NS], q1n, t_ert[:], start=True, stop=False)
                    nc.tensor.matmul(pt1[:, 0:NS], q2n, t_eitn[:], start=False, stop=True)
                    nc.tensor.matmul(pt1[:, NS:], q2n, t_ert[:], start=True, stop=False)
                    nc.tensor.matmul(pt1[:, NS:], q1n, t_eit[:], start=False, stop=True)
                    tts = tpool.tile([128, 2 * NS], f32, tag="tts")
                    tt1 = tpool.tile([1, 2 * NS], f32, tag="tt1")
                    nc.vector.tensor_copy(tts[:], ptt[:])
                    nc.vector.tensor_copy(tt1[:], pt1[:])

                    po = psO.tile([NS, NS], f32, tag="po")
                    nc.tensor.matmul(po[:], tts[:, 0:NS], t_wkr0[:], start=True, stop=False)
                    nc.tensor.matmul(po[:], tts[:, NS:], t_wkin0[:], start=False, stop=False)
                    nc.tensor.matmul(po[:], tt1[:, 0:NS], t_wkr1[:], start=False, stop=False)
                    nc.tensor.matmul(po[:], tt1[:, NS:], t_wkin1[:], start=False, stop=True)
                    nc.vector.tensor_copy(
                        t_stage[:, p * NS : (p + 1) * NS], po[:]
                    )

            nc.sync.dma_start(d_out[:], t_stage[:])
    return nc


def _kernel_device(x):
    sys.path.insert(0, "/opt/trn_rl_repo")
    sys.path.insert(0, "/opt/trn_rl_repo/concourse")
    from concourse.bass_utils import run_bass_kernel_spmd

    xs = _standardize(x)
    cwt, consts, wkr, wkin = _host_consts()
    nc = _build_program()

    in_maps = []
    for core in range(8):
        b, half = core // 2, core % 2
        crt, srt, ert, eit, eitn = consts[half]
        xt = np.ascontiguousarray(xs[b].transpose(2, 1, 0))  # [C, W, H]
        in_maps.append(
            {
                "xt": xt, "cwt": cwt, "crt": crt, "srt": srt,
                "ert": ert, "eit": eit, "eitn": eitn,
                "wkr": wkr, "wkin": wkin,
            }
        )
    res = run_bass_kernel_spmd(nc, in_maps, list(range(8)))
    out = np.empty((B, NS, NS, P), dtype=np.float32)
    for b in range(B):
        st = res.results[2 * b]["yout"] + res.results[2 * b + 1]["yout"]
        out[b] = st.reshape(NS, P, NS).transpose(0, 2, 1)
    return out


def _idft_mats():
    sy = np.arange(NS) - MS
    ky = np.arange(H)
    E = np.exp(2j * np.pi * np.outer(sy, ky) / H).astype(np.complex64)
    sx = np.arange(NS) - MS
    kx = np.arange(KX)
    w = np.full(KX, 2.0)
    w[0] = 1.0
    w[KX - 1] = 1.0
    Wk = (w[:, None] * np.exp(2j * np.pi * np.outer(kx, sx) / W) / NPIX).astype(
        np.complex64
    )
    return E, Wk


def _kernel_numpy(x):
    """BLAS host fallback."""
    xs = _standardize(x)
    ii, jj = np.triu_indices(C)
    E, Wk = _idft_mats()
    Er, Ei = np.ascontiguousarray(E.real), np.ascontiguousarray(E.imag)
    Wkr, Wki = np.ascontiguousarray(Wk.real), np.ascontiguousarray(Wk.imag)
    out = np.empty((B, NS, NS, P), dtype=np.float32)
    for b in range(B):
        xb = np.transpose(xs[b], (2, 0, 1))
        f = np.fft.rfft2(xb)
        fr = np.ascontiguousarray(f.real, dtype=np.float32)
        fi = np.ascontiguousarray(f.imag, dtype=np.float32)
        ccr = fr[ii] * fr[jj] + fi[ii] * fi[jj]
        cci = fi[ii] * fr[jj] - fr[ii] * fi[jj]
        ccr_m = ccr.transpose(1, 0, 2).reshape(H, P * KX)
        cci_m = cci.transpose(1, 0, 2).reshape(H, P * KX)
        Tr = Er @ ccr_m - Ei @ cci_m
        Ti = Er @ cci_m + Ei @ ccr_m
        Tr3 = Tr.reshape(NS, P, KX).transpose(1, 0, 2).reshape(P * NS, KX)
        Ti3 = Ti.reshape(NS, P, KX).transpose(1, 0, 2).reshape(P * NS, KX)
        o = Tr3 @ Wkr - Ti3 @ Wki
        out[b] = o.reshape(P, NS, NS).transpose(1, 2, 0)
    return out


def kernel(x):
    x = np.asarray(x, dtype=np.float32)
    try:
        return _kernel_device(x)
    except Exception as e:
        print(f"device path failed ({type(e).__name__}: {e}); numpy fallback",
              file=sys.stderr)
        return _kernel_numpy(x)
